# revision 1
# baseline (speedup 1.0000x reference)
"""HGNN encoder (2-layer hypergraph message passing) as an 8-core TRN2 Bass kernel.

Strategy (1D node partition, K-sharded G-matmuls):
  - Each core owns a contiguous shard of user nodes (U/8) and item nodes (I/8).
  - Layer matmuls user_hyper@user_emb / item_hyper@item_emb contract over nodes:
    each core multiplies its node-columns of the (host-pre-transposed) incidence
    slices against its node shard of the embeddings, producing a PARTIAL
    [64, G] message accumulated in PSUM; a [128, G] AllReduce (user+item packed)
    yields the full messages on every core.
  - The attention/update block (tiny) is computed redundantly on every core in
    a transposed [64, G] layout, which makes every matmul layout-natural.
  - full_hyper@msg contracts over G: each core computes its node shard with
    full_hyperT column-slices, output directly node-major -> no transposes of
    big tensors anywhere, and the layer-2 embeddings each core needs are
    exactly the node shard it just produced (no big all-gather).
"""

import numpy as np

U, I, G, D = 30000, 60000, 2000, 64
L = 2
NCORES = 8
UC, IC = U // NCORES, I // NCORES
GPAD = 2048  # G rounded to 512


def _ktiles(n, step=128):
    return [(s, min(step, n - s)) for s in range(0, n, step)]


GCH = _ktiles(G, 512)     # 4 free-dim chunks for matmul N<=512
GTL = _ktiles(G, 128)     # 16 partition tiles


def _build():
    import concourse.bacc as bacc
    import concourse.mybir as mybir
    import concourse.tile as tile
    from concourse import masks

    f32 = mybir.dt.float32
    nc = bacc.Bacc("TRN2", target_bir_lowering=False, debug=False,
                   num_devices=NCORES)

    def din(name, shape):
        return nc.dram_tensor(name, shape, f32, kind="ExternalInput").ap()

    def dout(name, shape):
        return nc.dram_tensor(name, shape, f32, kind="ExternalOutput").ap()

    ue = din("ue", [UC, D])
    ie = din("ie", [IC, D])
    uhT = din("uhT", [UC, G])
    ihT = din("ihT", [IC, G])
    fhTu = din("fhTu", [G, UC])
    fhTi = din("fhTi", [G, IC])
    gT = din("gT", [D, G])
    qc_w1 = din("qc_w1", [L, D, D])
    qc_b1 = din("qc_b1", [L, D])
    qc_w2 = din("qc_w2", [L, D, 1])
    user_w = din("user_w", [L, 2 * D, D])
    user_b = din("user_b", [L, D])
    item_w = din("item_w", [L, 2 * D, D])
    item_b = din("item_b", [L, D])

    final_u = dout("final_u", [UC, D])
    final_i = dout("final_i", [IC, D])
    final_he = dout("final_he", [G, D])

    import os
    dbg = bool(os.environ.get("KDBG"))
    if dbg:
        d_part = dout("d_part", [128, G])
        d_msgu = dout("d_msgu", [64, G])
        d_msgi = dout("d_msgi", [64, G])
        d_hu = dout("d_hu", [64, G])
        d_wu = dout("d_wu", [1, G])
        d_common = dout("d_common", [64, G])
        d_msgNT = dout("d_msgNT", [64, G])
        d_uiu = dout("d_uiu", [128, 30 * D])
        d_msgP = dout("d_msgP", [128, 16 * D])

    ukt = _ktiles(UC)   # 30 k-tiles for the user shard
    ikt = _ktiles(IC)   # 59 k-tiles for the item shard

    with tile.TileContext(nc) as tc:
        with (
            tc.tile_pool(name="const", bufs=1) as cpool,
            tc.tile_pool(name="resid", bufs=1) as rpool,
            tc.tile_pool(name="rhsA", bufs=3) as apool,
            tc.tile_pool(name="fhD", bufs=5) as dpool,
            tc.tile_pool(name="pb", bufs=2) as bpool,
            tc.tile_pool(name="outp", bufs=4) as opool,
            tc.tile_pool(name="psA", bufs=1, space="PSUM") as ps_a,
            tc.tile_pool(name="psD", bufs=1, space="PSUM") as ps_d,
            tc.tile_pool(name="psB", bufs=1, space="PSUM") as ps_b,
            tc.tile_pool(name="dram", bufs=1, space="DRAM") as drpool,
        ):
            # ---- constants / weights -------------------------------------
            ident = cpool.tile([128, 128], f32, tag="ident", name="ident")
            masks.make_identity(nc, ident[:])
            ones1 = cpool.tile([1, D], f32, tag="ones1", name="ones1")
            nc.vector.memset(ones1[:], 1.0)

            gT_s = cpool.tile([D, GPAD], f32, tag="gTs", name="gT_s")
            nc.sync.dma_start(gT_s[:, :G], gT[:, :])

            w1_s, w2_s, b1_s, uw_s, iw_s, bsum_s = [], [], [], [], [], []
            for l in range(L):
                w1 = cpool.tile([D, D], f32, tag=f"w1_{l}", name=f"w1_{l}")
                nc.sync.dma_start(w1[:], qc_w1[l])
                w1_s.append(w1)
                w2 = cpool.tile([D, 1], f32, tag=f"w2_{l}", name=f"w2_{l}")
                nc.sync.dma_start(w2[:], qc_w2[l])
                w2n = cpool.tile([D, 1], f32, tag=f"w2n_{l}", name=f"w2n_{l}")
                nc.scalar.mul(w2n[:], w2[:], -1.0)
                w2_s.append((w2, w2n))
                b1 = cpool.tile([D, 1], f32, tag=f"b1_{l}", name=f"b1_{l}")
                nc.sync.dma_start(b1[:], qc_b1[l].unsqueeze(1))
                b1_s.append(b1)
                uw0 = cpool.tile([D, D], f32, tag=f"uw0_{l}", name=f"uw0_{l}")
                nc.sync.dma_start(uw0[:], user_w[l][0:D, :])
                uw1 = cpool.tile([D, D], f32, tag=f"uw1_{l}", name=f"uw1_{l}")
                nc.sync.dma_start(uw1[:], user_w[l][D:2 * D, :])
                uw_s.append((uw0, uw1))
                iw0 = cpool.tile([D, D], f32, tag=f"iw0_{l}", name=f"iw0_{l}")
                nc.sync.dma_start(iw0[:], item_w[l][0:D, :])
                iw1 = cpool.tile([D, D], f32, tag=f"iw1_{l}", name=f"iw1_{l}")
                nc.sync.dma_start(iw1[:], item_w[l][D:2 * D, :])
                iw_s.append((iw0, iw1))
                ub = cpool.tile([D, 1], f32, tag=f"ub_{l}", name=f"ub_{l}")
                nc.sync.dma_start(ub[:], user_b[l].unsqueeze(1))
                ib = cpool.tile([D, 1], f32, tag=f"ib_{l}", name=f"ib_{l}")
                nc.sync.dma_start(ib[:], item_b[l].unsqueeze(1))
                bs = cpool.tile([D, 1], f32, tag=f"bs_{l}", name=f"bs_{l}")
                nc.vector.tensor_add(bs[:], ub[:], ib[:])
                bsum_s.append(bs)

            # ---- residents ----------------------------------------------
            # node embeddings / layer-1 outputs, [128, ntiles*64] tiled layout
            ue_res = rpool.tile([128, len(ukt) * D], f32, tag="ue_res", name="ue_res")
            ie_res = rpool.tile([128, len(ikt) * D], f32, tag="ie_res", name="ie_res")
            ui_u = rpool.tile([128, len(ukt) * D], f32, tag="ui_u", name="ui_u")
            ui_i = rpool.tile([128, len(ikt) * D], f32, tag="ui_i", name="ui_i")
            he_acc = rpool.tile([D, GPAD], f32, tag="he_acc", name="he_acc")
            for t, (k0, kk) in enumerate(ukt):
                nc.sync.dma_start(ue_res[:kk, t * D:(t + 1) * D], ue[k0:k0 + kk, :])
            for t, (k0, kk) in enumerate(ikt):
                nc.sync.dma_start(ie_res[:kk, t * D:(t + 1) * D], ie[k0:k0 + kk, :])
            nc.vector.tensor_copy(he_acc[:, :G], gT_s[:, :G])

            for l in range(L):
                # ==== Phase A: partial messages, K-sharded over nodes ====
                cc_in = drpool.tile([128, G], f32, tag=f"cc_in_{l}",
                                    name=f"cc_in_{l}")
                cc_out = drpool.tile([128, G], f32, tag=f"cc_out_{l}",
                                     addr_space="Shared", name=f"cc_out_{l}")
                for part, (hyT, kt, emb0, emb1) in enumerate([
                        (uhT, ukt, ue_res, ui_u), (ihT, ikt, ie_res, ui_i)]):
                    lhs_res = emb0 if l == 0 else emb1
                    ps_msg = ps_a.tile([64, GPAD], f32, tag="msgps",
                                       name=f"msgps_{l}_{part}")
                    nkt = len(kt)
                    for t, (k0, kk) in enumerate(kt):
                        rt = apool.tile([128, GPAD], f32, tag="rhsA",
                                        name=f"rhsA_{l}_{part}_{t}")
                        nc.sync.dma_start(rt[:kk, :G], hyT[k0:k0 + kk, :])
                        for (g0, gw) in GCH:
                            nc.tensor.matmul(
                                ps_msg[:, g0:g0 + gw],
                                lhsT=lhs_res[:kk, t * D:(t + 1) * D],
                                rhs=rt[:kk, g0:g0 + gw],
                                start=(t == 0), stop=(t == nkt - 1))
                    pdrain = bpool.tile([64, GPAD], f32, tag=f"pdrain{part}",
                                        bufs=1, name=f"pdrain_{l}_{part}")
                    nc.vector.tensor_copy(pdrain[:, :G], ps_msg[:, :G])
                    nc.sync.dma_start(cc_in[part * 64:(part + 1) * 64, :],
                                      pdrain[:, :G])

                # ==== AllReduce of [128, G] packed partials ====
                if dbg and l == 0:
                    nc.sync.dma_start(d_part[:], cc_in[:])
                nc.gpsimd.collective_compute(
                    "AllReduce", mybir.AluOpType.add,
                    ins=[cc_in.opt()], outs=[cc_out.opt()],
                    replica_groups=[list(range(NCORES))])
                msgT_u = bpool.tile([64, GPAD], f32, tag="msgT_u", bufs=1,
                                    name=f"msgT_u_{l}")
                msgT_i = bpool.tile([64, GPAD], f32, tag="msgT_i", bufs=1,
                                    name=f"msgT_i_{l}")
                nc.sync.dma_start(msgT_u[:, :G], cc_out[0:64, :])
                nc.sync.dma_start(msgT_i[:, :G], cc_out[64:128, :])
                if dbg and l == 0:
                    nc.sync.dma_start(d_msgu[:], msgT_u[:, :G])
                    nc.sync.dma_start(d_msgi[:], msgT_i[:, :G])

                # ==== Phase B: attention + node-update weights (full G) ====
                msgNT = bpool.tile([64, GPAD], f32, tag="msgNT", bufs=1,
                                   name=f"msgNT_{l}")
                for ci, (g0, gw) in enumerate(GCH):
                    sl = slice(g0, g0 + gw)
                    um = msgT_u[:, sl]
                    im = msgT_i[:, sl]
                    # h = tanh(msg @ W1 + b1), transposed layout
                    hu_ps = ps_b.tile([64, 512], f32, tag="pb", name=f"hu_{l}_{ci}")
                    nc.tensor.matmul(hu_ps[:, :gw], lhsT=w1_s[l][:], rhs=um,
                                     start=True, stop=True)
                    hu = bpool.tile([64, 512], f32, tag="hu", name=f"hus_{l}_{ci}")
                    nc.scalar.activation(hu[:, :gw], hu_ps[:, :gw],
                                         mybir.ActivationFunctionType.Tanh,
                                         bias=b1_s[l][:])
                    hi_ps = ps_b.tile([64, 512], f32, tag="pb", name=f"hi_{l}_{ci}")
                    nc.tensor.matmul(hi_ps[:, :gw], lhsT=w1_s[l][:], rhs=im,
                                     start=True, stop=True)
                    hi = bpool.tile([64, 512], f32, tag="hi", name=f"his_{l}_{ci}")
                    nc.scalar.activation(hi[:, :gw], hi_ps[:, :gw],
                                         mybir.ActivationFunctionType.Tanh,
                                         bias=b1_s[l][:])
                    # attention logit diff a_u - a_i accumulated in one bank;
                    # softmax over 2 == sigmoid(+-diff)
                    ad_ps = ps_b.tile([1, 512], f32, tag="pb", name=f"ad_{l}_{ci}")
                    nc.tensor.matmul(ad_ps[:, :gw], lhsT=w2_s[l][0][:],
                                     rhs=hu[:, :gw], start=True, stop=False)
                    nc.tensor.matmul(ad_ps[:, :gw], lhsT=w2_s[l][1][:],
                                     rhs=hi[:, :gw], start=False, stop=True)
                    wu = bpool.tile([1, 512], f32, tag="wud", bufs=3,
                                    name=f"wu_{l}_{ci}")
                    nc.scalar.activation(wu[:, :gw], ad_ps[:, :gw],
                                         mybir.ActivationFunctionType.Sigmoid)
                    wi = bpool.tile([1, 512], f32, tag="wud", bufs=3,
                                    name=f"wi_{l}_{ci}")
                    nc.scalar.activation(wi[:, :gw], ad_ps[:, :gw],
                                         mybir.ActivationFunctionType.Sigmoid,
                                         scale=-1.0)
                    # broadcast weights across 64 partitions via outer product
                    wub_ps = ps_b.tile([64, 512], f32, tag="pb", name=f"wub_{l}_{ci}")
                    nc.tensor.matmul(wub_ps[:, :gw], lhsT=ones1[:], rhs=wu[:, :gw],
                                     start=True, stop=True)
                    wib_ps = ps_b.tile([64, 512], f32, tag="pb", name=f"wib_{l}_{ci}")
                    nc.tensor.matmul(wib_ps[:, :gw], lhsT=ones1[:], rhs=wi[:, :gw],
                                     start=True, stop=True)
                    common = bpool.tile([64, 512], f32, tag="common",
                                        name=f"common_{l}_{ci}")
                    tmpc = bpool.tile([64, 512], f32, tag="tmpc",
                                      name=f"tmpc_{l}_{ci}")
                    nc.vector.tensor_mul(common[:, :gw], um, wub_ps[:, :gw])
                    nc.vector.tensor_mul(tmpc[:, :gw], im, wib_ps[:, :gw])
                    nc.vector.tensor_add(common[:, :gw], common[:, :gw],
                                         tmpc[:, :gw])
                    dfu = bpool.tile([64, 512], f32, tag="dfu", name=f"dfu_{l}_{ci}")
                    dfi = bpool.tile([64, 512], f32, tag="dfi", name=f"dfi_{l}_{ci}")
                    nc.vector.tensor_sub(dfu[:, :gw], um, common[:, :gw])
                    nc.vector.tensor_sub(dfi[:, :gw], im, common[:, :gw])
                    # u2+i2 accumulated: [diff,g] @ user_w + [diff,g] @ item_w
                    o2_ps = ps_b.tile([64, 512], f32, tag="pb", name=f"o2_{l}_{ci}")
                    nc.tensor.matmul(o2_ps[:, :gw], lhsT=uw_s[l][0][:],
                                     rhs=dfu[:, :gw], start=True, stop=False)
                    nc.tensor.matmul(o2_ps[:, :gw], lhsT=uw_s[l][1][:],
                                     rhs=gT_s[:, sl], start=False, stop=False)
                    nc.tensor.matmul(o2_ps[:, :gw], lhsT=iw_s[l][0][:],
                                     rhs=dfi[:, :gw], start=False, stop=False)
                    nc.tensor.matmul(o2_ps[:, :gw], lhsT=iw_s[l][1][:],
                                     rhs=gT_s[:, sl], start=False, stop=True)
                    # msg = u2 + i2 + (user_b+item_b) + common
                    nc.vector.scalar_tensor_tensor(
                        msgNT[:, sl], o2_ps[:, :gw], bsum_s[l][:], common[:, :gw],
                        op0=mybir.AluOpType.add, op1=mybir.AluOpType.add)
                    nc.vector.tensor_add(he_acc[:, sl], he_acc[:, sl], msgNT[:, sl])
                    if dbg and l == 0:
                        nc.sync.dma_start(d_hu[:, sl], hu[:, :gw])
                        nc.sync.dma_start(d_wu[:, sl], wu[:, :gw])
                        nc.sync.dma_start(d_common[:, sl], common[:, :gw])
                        nc.sync.dma_start(d_msgNT[:, sl], msgNT[:, sl])

                # ==== Phase C: transpose msg to [G, 64] node-update layout ====
                msgP = rpool.tile([128, len(GTL) * D], f32, tag="msgP",
                                  name=f"msgP_{l}")
                for t, (g0, gg) in enumerate(GTL):
                    tp_ps = ps_b.tile([128, 64], f32, tag="pb", name=f"tp_{l}_{t}")
                    nc.tensor.transpose(tp_ps[:gg, :], msgNT[:, g0:g0 + gg],
                                        ident[:64, :64])
                    nc.vector.tensor_copy(msgP[:gg, t * D:(t + 1) * D],
                                          tp_ps[:gg, :])
                if dbg and l == 0:
                    nc.sync.dma_start(d_msgP[:], msgP[:])

                # ==== Phase D: node_out = full_hyper @ msg (node shard) ====
                # each 128-node sub accumulates in its own PSUM bank: groups
                # interleaved across banks are fine, within one bank they
                # are not (hw/walrus accumulation state is per-bank).
                NCH = 384
                for part, (fhT, nn_total, emb_res, ui_res, fout) in enumerate([
                        (fhTu, UC, ue_res, ui_u, final_u),
                        (fhTi, IC, ie_res, ui_i, final_i)]):
                    for ch, (n0, nw) in enumerate(_ktiles(nn_total, NCH)):
                        subs = _ktiles(nw)
                        pds = [ps_d.tile([128, D], f32, tag=f"pd{s}",
                                         name=f"pd_{l}_{part}_{ch}_{s}")
                               for s in range(len(subs))]
                        ngt = len(GTL)
                        for t, (g0, gg) in enumerate(GTL):
                            ft = dpool.tile([128, NCH], f32, tag="fh",
                                            name=f"fh_{l}_{part}_{ch}_{t}")
                            nc.sync.dma_start(ft[:gg, :nw],
                                              fhT[g0:g0 + gg, n0:n0 + nw])
                            for s, (s0, ss) in enumerate(subs):
                                nc.tensor.matmul(
                                    pds[s][:ss, :],
                                    lhsT=ft[:gg, s0:s0 + ss],
                                    rhs=msgP[:gg, t * D:(t + 1) * D],
                                    start=(t == 0), stop=(t == ngt - 1))
                        for s, (s0, ss) in enumerate(subs):
                            ti = (n0 + s0) // 128
                            tsl = slice(ti * D, (ti + 1) * D)
                            psl = pds[s][:ss, :]
                            if l == 0:
                                nc.vector.tensor_copy(ui_res[:ss, tsl], psl)
                                nc.vector.tensor_add(emb_res[:ss, tsl],
                                                     emb_res[:ss, tsl], psl)
                            else:
                                fo = opool.tile([128, D], f32, tag="fo",
                                                name=f"fo_{l}_{part}_{ch}_{s}")
                                nc.vector.tensor_add(fo[:ss, :],
                                                     emb_res[:ss, tsl], psl)
                                nc.sync.dma_start(fout[n0 + s0:n0 + s0 + ss, :],
                                                  fo[:ss, :])

            if dbg:
                nc.sync.dma_start(d_uiu[:], ui_u[:])

            # ==== final_he = group_emb + msg1 + msg2, transpose out ====
            for t, (g0, gg) in enumerate(GTL):
                tp_ps = ps_b.tile([128, 64], f32, tag="pb", name=f"he_t_{t}")
                nc.tensor.transpose(tp_ps[:gg, :], he_acc[:, g0:g0 + gg],
                                    ident[:64, :64])
                ho = opool.tile([128, D], f32, tag="ho", name=f"ho_{t}")
                nc.vector.tensor_copy(ho[:gg, :], tp_ps[:gg, :])
                nc.sync.dma_start(final_he[g0:g0 + gg, :], ho[:gg, :])

    nc.compile()
    return nc


_NC_CACHE = {}


def _get_nc():
    if "nc" not in _NC_CACHE:
        _NC_CACHE["nc"] = _build()
    return _NC_CACHE["nc"]


def make_in_maps(user_emb, item_emb, group_emb, user_hyper, item_hyper,
                 full_hyper, qc_w1, qc_b1, qc_w2, user_w, user_b, item_w,
                 item_b):
    f = np.float32
    rep = {
        "gT": np.ascontiguousarray(np.asarray(group_emb, f).T),
        "qc_w1": np.asarray(qc_w1, f), "qc_b1": np.asarray(qc_b1, f),
        "qc_w2": np.asarray(qc_w2, f),
        "user_w": np.asarray(user_w, f), "user_b": np.asarray(user_b, f),
        "item_w": np.asarray(item_w, f), "item_b": np.asarray(item_b, f),
    }
    user_hyper = np.asarray(user_hyper, f)
    item_hyper = np.asarray(item_hyper, f)
    full_hyper = np.asarray(full_hyper, f)
    user_emb = np.asarray(user_emb, f)
    item_emb = np.asarray(item_emb, f)
    in_maps = []
    for c in range(NCORES):
        us = slice(c * UC, (c + 1) * UC)
        isl = slice(c * IC, (c + 1) * IC)
        m = dict(rep)
        m["ue"] = np.ascontiguousarray(user_emb[us])
        m["ie"] = np.ascontiguousarray(item_emb[isl])
        m["uhT"] = np.ascontiguousarray(user_hyper[:, us].T)
        m["ihT"] = np.ascontiguousarray(item_hyper[:, isl].T)
        m["fhTu"] = np.ascontiguousarray(full_hyper[us, :].T)
        m["fhTi"] = np.ascontiguousarray(
            full_hyper[U + c * IC:U + (c + 1) * IC, :].T)
        in_maps.append(m)
    return in_maps


def assemble(results):
    out = np.empty((U + I + G, D), np.float32)
    for c in range(NCORES):
        out[c * UC:(c + 1) * UC] = results[c]["final_u"]
        out[U + c * IC:U + (c + 1) * IC] = results[c]["final_i"]
    out[U + I:] = results[0]["final_he"]
    return out


def kernel(user_emb, item_emb, group_emb, user_hyper, item_hyper, full_hyper,
           qc_w1, qc_b1, qc_w2, user_w, user_b, item_w, item_b,
           num_users=U, num_items=I):
    from concourse.bass_utils import run_bass_kernel_spmd
    nc = _get_nc()
    in_maps = make_in_maps(user_emb, item_emb, group_emb, user_hyper,
                           item_hyper, full_hyper, qc_w1, qc_b1, qc_w2,
                           user_w, user_b, item_w, item_b)
    res = run_bass_kernel_spmd(nc, in_maps, list(range(NCORES)))
    return assemble(res.results)



# revision 5
# speedup vs baseline: 1.2619x; 1.2619x over previous
"""HGNN encoder (2-layer hypergraph message passing) as an 8-core TRN2 Bass kernel.

Strategy (1D node partition, K-sharded G-matmuls, bf16 streaming):
  - Each core owns a contiguous shard of user nodes (U/8) and item nodes (I/8).
  - All large tensors (incidence matrices, embeddings) are cast to bf16 on the
    host: halves HBM traffic and runs the PE at 1 cycle/row instead of 4.
  - Layer matmuls user_hyper@user_emb / item_hyper@item_emb contract over
    nodes: each core multiplies its node-rows of the (host-pre-transposed,
    host-padded) incidence slices against its node shard of the embeddings,
    producing a PARTIAL [64, G] message in PSUM; a [128, G] bf16 AllReduce
    (user+item packed) yields the full messages on every core.
  - The attention/update block (tiny) is computed redundantly on every core in
    a transposed [64, G] layout (bf16 matmuls, fp32 PSUM).
  - full_hyper@msg contracts over G: each core computes its node shard with
    host-padded fhT column-slices, output directly node-major.  fhT tiles for
    the first chunks are prefetched before the AllReduce so the DMA queues
    never idle.  The residual stream (init emb + layer outputs) accumulates
    in fp32.
"""

import numpy as np

U, I, G, D = 30000, 60000, 2000, 64
L = 2
NCORES = 8
UC, IC = U // NCORES, I // NCORES        # 3750, 7500
NKU = (UC + 127) // 128                  # 30 k-tiles (user shard)
NKI = (IC + 127) // 128                  # 59 k-tiles (item shard)
UCP, ICP = NKU * 128, NKI * 128          # host-padded shard sizes
GPAD = 2048
GT = GPAD // 128                         # 16 g-tiles
NCH = 768                                # phase-D node chunk (6 PSUM banks)


def _ch(n, step):
    return [(s, min(step, n - s)) for s in range(0, n, step)]


GCH = _ch(G, 512)                        # 4 free-dim chunks for matmul N<=512


def _build():
    import concourse.bacc as bacc
    import concourse.mybir as mybir
    import concourse.tile as tile
    from concourse import masks

    f32 = mybir.dt.float32
    bf16 = mybir.dt.bfloat16
    nc = bacc.Bacc("TRN2", target_bir_lowering=False, debug=False,
                   num_devices=NCORES)

    def din(name, shape, dt=f32):
        return nc.dram_tensor(name, shape, dt, kind="ExternalInput").ap()

    def dout(name, shape):
        return nc.dram_tensor(name, shape, f32, kind="ExternalOutput").ap()

    ue = din("ue", [UCP, D], bf16)
    ie = din("ie", [ICP, D], bf16)
    uhT = din("uhT", [UCP, G], bf16)
    ihT = din("ihT", [ICP, G], bf16)
    fhTu = din("fhTu", [GPAD, UC], bf16)
    fhTi = din("fhTi", [GPAD, IC], bf16)
    gT = din("gT", [D, G])
    qc_w1 = din("qc_w1", [L, D, D])
    qc_b1 = din("qc_b1", [L, D])
    qc_w2 = din("qc_w2", [L, D, 1])
    user_w = din("user_w", [L, 2 * D, D])
    user_b = din("user_b", [L, D])
    item_w = din("item_w", [L, 2 * D, D])
    item_b = din("item_b", [L, D])

    final_u = dout("final_u", [UC, D])
    final_i = dout("final_i", [IC, D])
    final_he = dout("final_he", [G, D])

    ujobs = [(0, n0, nw) for (n0, nw) in _ch(UC, NCH)]
    ijobs = [(1, n0, nw) for (n0, nw) in _ch(IC, NCH)]
    jobs = ujobs + ijobs
    PF = 2                               # fh chunks prefetched before the AR

    with tile.TileContext(nc) as tc:
        with (
            tc.tile_pool(name="const", bufs=1) as cpool,
            tc.tile_pool(name="resid", bufs=1) as rpool,
            tc.tile_pool(name="rhsA", bufs=4) as apool,
            tc.tile_pool(name="fhD", bufs=3) as dpool,
            tc.tile_pool(name="pb", bufs=2) as bpool,
            tc.tile_pool(name="outp", bufs=4) as opool,
            tc.tile_pool(name="psB", bufs=2, space="PSUM") as ps_b,
            tc.tile_pool(name="dram", bufs=1, space="DRAM") as drpool,
        ):
            # ---- constants / weights -------------------------------------
            ident = cpool.tile([128, 128], f32, tag="ident", name="ident")
            masks.make_identity(nc, ident[:])
            ones1b = cpool.tile([1, D], bf16, tag="ones1b", name="ones1b")
            nc.vector.memset(ones1b[:], 1.0)

            gT_s = cpool.tile([D, GPAD], f32, tag="gTs", name="gT_s")
            nc.sync.dma_start(gT_s[:, :G], gT[:, :])
            gTb = cpool.tile([D, GPAD], bf16, tag="gTb", name="gTb")
            nc.vector.memset(gTb[:, G:], 0.0)
            nc.vector.tensor_copy(gTb[:, :G], gT_s[:, :G])

            w1_s, w2_s, b1_s, uw_s, iw_s, bsum_s = [], [], [], [], [], []
            for l in range(L):
                stage = cpool.tile([2 * D, D], f32, tag=f"wst_{l}",
                                   name=f"wst_{l}")
                w1 = cpool.tile([D, D], bf16, tag=f"w1_{l}", name=f"w1_{l}")
                nc.sync.dma_start(stage[:D, :], qc_w1[l])
                nc.vector.tensor_copy(w1[:], stage[:D, :])
                w1_s.append(w1)
                w2f = cpool.tile([D, 1], f32, tag=f"w2f_{l}", name=f"w2f_{l}")
                nc.sync.dma_start(w2f[:], qc_w2[l])
                w2 = cpool.tile([D, 1], bf16, tag=f"w2_{l}", name=f"w2_{l}")
                nc.vector.tensor_copy(w2[:], w2f[:])
                w2n = cpool.tile([D, 1], bf16, tag=f"w2n_{l}", name=f"w2n_{l}")
                nc.vector.tensor_scalar_mul(w2n[:], w2f[:], -1.0)
                w2_s.append((w2, w2n))
                b1 = cpool.tile([D, 1], f32, tag=f"b1_{l}", name=f"b1_{l}")
                nc.sync.dma_start(b1[:], qc_b1[l].unsqueeze(1))
                b1_s.append(b1)
                st2 = cpool.tile([2 * D, D], f32, tag=f"wst2_{l}",
                                 name=f"wst2_{l}")
                nc.sync.dma_start(st2[:], user_w[l])
                uw0 = cpool.tile([D, D], bf16, tag=f"uw0_{l}", name=f"uw0_{l}")
                nc.vector.tensor_copy(uw0[:], st2[:D, :])
                uw1 = cpool.tile([D, D], bf16, tag=f"uw1_{l}", name=f"uw1_{l}")
                nc.vector.tensor_copy(uw1[:], st2[D:, :])
                uw_s.append((uw0, uw1))
                nc.sync.dma_start(stage[:], item_w[l])
                iw0 = cpool.tile([D, D], bf16, tag=f"iw0_{l}", name=f"iw0_{l}")
                nc.vector.tensor_copy(iw0[:], stage[:D, :])
                iw1 = cpool.tile([D, D], bf16, tag=f"iw1_{l}", name=f"iw1_{l}")
                nc.vector.tensor_copy(iw1[:], stage[D:, :])
                iw_s.append((iw0, iw1))
                ub = cpool.tile([D, 1], f32, tag=f"ub_{l}", name=f"ub_{l}")
                nc.sync.dma_start(ub[:], user_b[l].unsqueeze(1))
                ib = cpool.tile([D, 1], f32, tag=f"ib_{l}", name=f"ib_{l}")
                nc.sync.dma_start(ib[:], item_b[l].unsqueeze(1))
                bs = cpool.tile([D, 1], f32, tag=f"bs_{l}", name=f"bs_{l}")
                nc.vector.tensor_add(bs[:], ub[:], ib[:])
                bsum_s.append(bs)

            # ---- residents ----------------------------------------------
            ue_res = rpool.tile([128, NKU * D], bf16, tag="ue_res",
                                name="ue_res")
            ie_res = rpool.tile([128, NKI * D], bf16, tag="ie_res",
                                name="ie_res")
            ui_u = rpool.tile([128, NKU * D], bf16, tag="ui_u", name="ui_u")
            ui_i = rpool.tile([128, NKI * D], bf16, tag="ui_i", name="ui_i")
            acc_u = rpool.tile([128, NKU * D], f32, tag="acc_u", name="acc_u")
            acc_i = rpool.tile([128, NKI * D], f32, tag="acc_i", name="acc_i")
            he_acc = rpool.tile([D, GPAD], f32, tag="he_acc", name="he_acc")
            msgNT = rpool.tile([D, GPAD], f32, tag="msgNT", name="msgNT")
            mTu = rpool.tile([D, GPAD], bf16, tag="mTu", name="mTu")
            mTi = rpool.tile([D, GPAD], bf16, tag="mTi", name="mTi")
            msgP = rpool.tile([128, GT * D], bf16, tag="msgP", name="msgP")

            for t in range(NKU):
                nc.sync.dma_start(ue_res[:, t * D:(t + 1) * D],
                                  ue[t * 128:(t + 1) * 128, :])
            for t in range(NKI):
                nc.sync.dma_start(ie_res[:, t * D:(t + 1) * D],
                                  ie[t * 128:(t + 1) * 128, :])
            nc.vector.memset(ui_u[:], 0.0)
            nc.vector.memset(ui_i[:], 0.0)
            nc.vector.tensor_copy(acc_u[:], ue_res[:])
            nc.vector.tensor_copy(acc_i[:], ie_res[:])
            nc.vector.tensor_copy(he_acc[:, :G], gT_s[:, :G])
            nc.vector.memset(msgNT[:, G:], 0.0)
            nc.vector.memset(mTu[:, G:], 0.0)
            nc.vector.memset(mTi[:, G:], 0.0)

            pf = {}                       # job idx -> prefetched fh tile

            def load_fh(j):
                part, n0, nw = jobs[j]
                src = fhTu if part == 0 else fhTi
                ft = dpool.tile([128, GT * NCH], bf16, tag="fh",
                                name=f"fh_{j}")
                for t in range(GT):
                    nc.sync.dma_start(ft[:, t * NCH:t * NCH + nw],
                                      src[t * 128:(t + 1) * 128, n0:n0 + nw])
                pf[j] = ft

            for l in range(L):
                lhs_u = ue_res if l == 0 else ui_u
                lhs_i = ie_res if l == 0 else ui_i
                cc_in = drpool.tile([128, G], bf16, tag=f"cc_in_{l}",
                                    name=f"cc_in_{l}")
                cc_out = drpool.tile([128, G], bf16, tag=f"cc_out_{l}",
                                     addr_space="Shared", name=f"cc_out_{l}")

                # ==== Phase A: partial messages, K-sharded over nodes ====
                with tc.tile_pool(name=f"psA{l}", bufs=1,
                                  space="PSUM") as ps_a:
                    for part, (hyT, nkt, lhs) in enumerate([
                            (uhT, NKU, lhs_u), (ihT, NKI, lhs_i)]):
                        ps_msg = ps_a.tile([64, GPAD], f32, tag="msgps",
                                           name=f"msgps_{l}_{part}")
                        for t in range(nkt):
                            rt = apool.tile([128, GPAD], bf16, tag="rhsA",
                                            name=f"rhsA_{l}_{part}_{t}")
                            nc.sync.dma_start(rt[:, :G],
                                              hyT[t * 128:(t + 1) * 128, :])
                            for (g0, gw) in GCH:
                                nc.tensor.matmul(
                                    ps_msg[:, g0:g0 + gw],
                                    lhsT=lhs[:, t * D:(t + 1) * D],
                                    rhs=rt[:, g0:g0 + gw],
                                    start=(t == 0), stop=(t == nkt - 1))
                        pdrain = bpool.tile([64, G], bf16, tag="pdrain",
                                            bufs=2, name=f"pdrain_{l}_{part}")
                        nc.vector.tensor_copy(pdrain[:], ps_msg[:, :G])
                        nc.sync.dma_start(
                            cc_in[part * 64:(part + 1) * 64, :], pdrain[:])

                # prefetch the first fh chunks so DMA stays busy over the AR
                for j in range(PF):
                    load_fh(j)

                # ==== AllReduce of [128, G] packed partials (bf16) ====
                nc.gpsimd.collective_compute(
                    "AllReduce", mybir.AluOpType.add,
                    ins=[cc_in.opt()], outs=[cc_out.opt()],
                    replica_groups=[list(range(NCORES))])
                nc.sync.dma_start(mTu[:, :G], cc_out[0:64, :])
                nc.sync.dma_start(mTi[:, :G], cc_out[64:128, :])

                # ==== Phase B: attention + node-update weights (full G) ====
                for ci, (g0, gw) in enumerate(GCH):
                    sl = slice(g0, g0 + gw)
                    um = mTu[:, sl]
                    im = mTi[:, sl]
                    hu_ps = ps_b.tile([64, 512], f32, tag="pb",
                                      name=f"hu_{l}_{ci}")
                    nc.tensor.matmul(hu_ps[:, :gw], lhsT=w1_s[l][:], rhs=um,
                                     start=True, stop=True)
                    hu = bpool.tile([64, 512], bf16, tag="hu",
                                    name=f"hus_{l}_{ci}")
                    nc.scalar.activation(hu[:, :gw], hu_ps[:, :gw],
                                         mybir.ActivationFunctionType.Tanh,
                                         bias=b1_s[l][:])
                    hi_ps = ps_b.tile([64, 512], f32, tag="pb",
                                      name=f"hi_{l}_{ci}")
                    nc.tensor.matmul(hi_ps[:, :gw], lhsT=w1_s[l][:], rhs=im,
                                     start=True, stop=True)
                    hi = bpool.tile([64, 512], bf16, tag="hi",
                                    name=f"his_{l}_{ci}")
                    nc.scalar.activation(hi[:, :gw], hi_ps[:, :gw],
                                         mybir.ActivationFunctionType.Tanh,
                                         bias=b1_s[l][:])
                    # attention logit diff a_u - a_i accumulated in one bank;
                    # softmax over 2 == sigmoid(+-diff)
                    ad_ps = ps_b.tile([1, 512], f32, tag="pb",
                                      name=f"ad_{l}_{ci}")
                    nc.tensor.matmul(ad_ps[:, :gw], lhsT=w2_s[l][0][:],
                                     rhs=hu[:, :gw], start=True, stop=False)
                    nc.tensor.matmul(ad_ps[:, :gw], lhsT=w2_s[l][1][:],
                                     rhs=hi[:, :gw], start=False, stop=True)
                    wu = bpool.tile([1, 512], bf16, tag="wud", bufs=3,
                                    name=f"wu_{l}_{ci}")
                    nc.scalar.activation(wu[:, :gw], ad_ps[:, :gw],
                                         mybir.ActivationFunctionType.Sigmoid)
                    wi = bpool.tile([1, 512], bf16, tag="wud", bufs=3,
                                    name=f"wi_{l}_{ci}")
                    nc.scalar.activation(wi[:, :gw], ad_ps[:, :gw],
                                         mybir.ActivationFunctionType.Sigmoid,
                                         scale=-1.0)
                    # broadcast weights across 64 partitions via outer product
                    wub_ps = ps_b.tile([64, 512], f32, tag="pb",
                                       name=f"wub_{l}_{ci}")
                    nc.tensor.matmul(wub_ps[:, :gw], lhsT=ones1b[:],
                                     rhs=wu[:, :gw], start=True, stop=True)
                    wib_ps = ps_b.tile([64, 512], f32, tag="pb",
                                       name=f"wib_{l}_{ci}")
                    nc.tensor.matmul(wib_ps[:, :gw], lhsT=ones1b[:],
                                     rhs=wi[:, :gw], start=True, stop=True)
                    common = bpool.tile([64, 512], f32, tag="common",
                                        name=f"common_{l}_{ci}")
                    tmpc = bpool.tile([64, 512], f32, tag="tmpc",
                                      name=f"tmpc_{l}_{ci}")
                    nc.vector.tensor_mul(common[:, :gw], um, wub_ps[:, :gw])
                    nc.vector.tensor_mul(tmpc[:, :gw], im, wib_ps[:, :gw])
                    nc.vector.tensor_add(common[:, :gw], common[:, :gw],
                                         tmpc[:, :gw])
                    dfu = bpool.tile([64, 512], bf16, tag="dfu",
                                     name=f"dfu_{l}_{ci}")
                    dfi = bpool.tile([64, 512], bf16, tag="dfi",
                                     name=f"dfi_{l}_{ci}")
                    nc.vector.tensor_sub(dfu[:, :gw], um, common[:, :gw])
                    nc.vector.tensor_sub(dfi[:, :gw], im, common[:, :gw])
                    # u2+i2 accumulated: [diff,g] @ user_w + [diff,g] @ item_w
                    o2_ps = ps_b.tile([64, 512], f32, tag="pb",
                                      name=f"o2_{l}_{ci}")
                    nc.tensor.matmul(o2_ps[:, :gw], lhsT=uw_s[l][0][:],
                                     rhs=dfu[:, :gw], start=True, stop=False)
                    nc.tensor.matmul(o2_ps[:, :gw], lhsT=uw_s[l][1][:],
                                     rhs=gTb[:, sl], start=False, stop=False)
                    nc.tensor.matmul(o2_ps[:, :gw], lhsT=iw_s[l][0][:],
                                     rhs=dfi[:, :gw], start=False, stop=False)
                    nc.tensor.matmul(o2_ps[:, :gw], lhsT=iw_s[l][1][:],
                                     rhs=gTb[:, sl], start=False, stop=True)
                    # msg = u2 + i2 + (user_b+item_b) + common
                    nc.vector.scalar_tensor_tensor(
                        msgNT[:, sl], o2_ps[:, :gw], bsum_s[l][:],
                        common[:, :gw],
                        op0=mybir.AluOpType.add, op1=mybir.AluOpType.add)
                    nc.vector.tensor_add(he_acc[:, sl], he_acc[:, sl],
                                         msgNT[:, sl])

                # ==== Phase C: transpose msg to [G, 64] node-major layout ====
                for t in range(GT):
                    tp_ps = ps_b.tile([128, 64], f32, tag="pb",
                                      name=f"tp_{l}_{t}")
                    nc.tensor.transpose(tp_ps[:], msgNT[:, t * 128:(t + 1) * 128],
                                        ident[:64, :64])
                    nc.vector.tensor_copy(msgP[:, t * D:(t + 1) * D],
                                          tp_ps[:])

                # ==== Phase D: node_out = full_hyper @ msg (node shard) ====
                with tc.tile_pool(name=f"psD{l}", bufs=1,
                                  space="PSUM") as ps_d:
                    for j, (part, n0, nw) in enumerate(jobs):
                        if j + PF < len(jobs):
                            load_fh(j + PF)
                        ft = pf.pop(j)
                        subs = _ch(nw, 128)
                        pds = [ps_d.tile([128, D], f32, tag=f"pd{s}",
                                         name=f"pd_{l}_{j}_{s}")
                               for s in range(len(subs))]
                        for t in range(GT):
                            for s, (s0, ss) in enumerate(subs):
                                nc.tensor.matmul(
                                    pds[s][:ss, :],
                                    lhsT=ft[:, t * NCH + s0:t * NCH + s0 + ss],
                                    rhs=msgP[:, t * D:(t + 1) * D],
                                    start=(t == 0), stop=(t == GT - 1))
                        acc = acc_u if part == 0 else acc_i
                        ui_res = ui_u if part == 0 else ui_i
                        fout = final_u if part == 0 else final_i
                        for s, (s0, ss) in enumerate(subs):
                            ti = (n0 + s0) // 128
                            tsl = slice(ti * D, (ti + 1) * D)
                            psl = pds[s][:ss, :]
                            if l == 0:
                                nc.vector.tensor_copy(ui_res[:ss, tsl], psl)
                                nc.vector.tensor_add(acc[:ss, tsl],
                                                     acc[:ss, tsl], psl)
                            else:
                                fo = opool.tile([128, D], f32, tag="fo",
                                                name=f"fo_{l}_{j}_{s}")
                                nc.vector.tensor_add(fo[:ss, :],
                                                     acc[:ss, tsl], psl)
                                nc.sync.dma_start(
                                    fout[n0 + s0:n0 + s0 + ss, :], fo[:ss, :])

            # ==== final_he = group_emb + msg1 + msg2, transpose out ====
            for (g0, gg) in _ch(G, 128):
                tp_ps = ps_b.tile([128, 64], f32, tag="pb", name=f"he_t_{g0}")
                nc.tensor.transpose(tp_ps[:gg, :], he_acc[:, g0:g0 + gg],
                                    ident[:64, :64])
                ho = opool.tile([128, D], f32, tag="ho", name=f"ho_{g0}")
                nc.vector.tensor_copy(ho[:gg, :], tp_ps[:gg, :])
                nc.sync.dma_start(final_he[g0:g0 + gg, :], ho[:gg, :])

    nc.compile()
    return nc


_NC_CACHE = {}


def _get_nc():
    if "nc" not in _NC_CACHE:
        _NC_CACHE["nc"] = _build()
    return _NC_CACHE["nc"]


def _pad_rows(a, n):
    out = np.zeros((n,) + a.shape[1:], a.dtype)
    out[:a.shape[0]] = a
    return out


def make_in_maps(user_emb, item_emb, group_emb, user_hyper, item_hyper,
                 full_hyper, qc_w1, qc_b1, qc_w2, user_w, user_b, item_w,
                 item_b):
    import ml_dtypes
    bf = ml_dtypes.bfloat16
    f = np.float32
    rep = {
        "gT": np.ascontiguousarray(np.asarray(group_emb, f).T),
        "qc_w1": np.asarray(qc_w1, f), "qc_b1": np.asarray(qc_b1, f),
        "qc_w2": np.asarray(qc_w2, f),
        "user_w": np.asarray(user_w, f), "user_b": np.asarray(user_b, f),
        "item_w": np.asarray(item_w, f), "item_b": np.asarray(item_b, f),
    }
    ue_b = np.asarray(user_emb, f).astype(bf)
    ie_b = np.asarray(item_emb, f).astype(bf)
    uhT_all = np.ascontiguousarray(np.asarray(user_hyper, f).T.astype(bf))
    ihT_all = np.ascontiguousarray(np.asarray(item_hyper, f).T.astype(bf))
    fhT_all = np.ascontiguousarray(np.asarray(full_hyper, f).T.astype(bf))
    in_maps = []
    for c in range(NCORES):
        us = slice(c * UC, (c + 1) * UC)
        isl = slice(c * IC, (c + 1) * IC)
        m = dict(rep)
        m["ue"] = _pad_rows(ue_b[us], UCP)
        m["ie"] = _pad_rows(ie_b[isl], ICP)
        m["uhT"] = _pad_rows(uhT_all[us], UCP)
        m["ihT"] = _pad_rows(ihT_all[isl], ICP)
        m["fhTu"] = _pad_rows(np.ascontiguousarray(fhT_all[:, us]), GPAD)
        m["fhTi"] = _pad_rows(
            np.ascontiguousarray(fhT_all[:, U + c * IC:U + (c + 1) * IC]),
            GPAD)
        in_maps.append(m)
    return in_maps


def assemble(results):
    out = np.empty((U + I + G, D), np.float32)
    for c in range(NCORES):
        out[c * UC:(c + 1) * UC] = results[c]["final_u"]
        out[U + c * IC:U + (c + 1) * IC] = results[c]["final_i"]
    out[U + I:] = results[0]["final_he"]
    return out


def kernel(user_emb, item_emb, group_emb, user_hyper, item_hyper, full_hyper,
           qc_w1, qc_b1, qc_w2, user_w, user_b, item_w, item_b,
           num_users=U, num_items=I):
    from concourse.bass_utils import run_bass_kernel_spmd
    nc = _get_nc()
    in_maps = make_in_maps(user_emb, item_emb, group_emb, user_hyper,
                           item_hyper, full_hyper, qc_w1, qc_b1, qc_w2,
                           user_w, user_b, item_w, item_b)
    res = run_bass_kernel_spmd(nc, in_maps, list(range(NCORES)))
    return assemble(res.results)
